# revision 112
# baseline (speedup 1.0000x reference)
"""Trainium2 Bass kernel for nn_BasicalBlock (VMamba-style SS4D block).

8 cores, core c = (b, k): b = c//4 (batch), k = c%4 (scan direction). The
host pre-permutes each core's x into its direction's scan order ("view
coords"), so phase-1 (LayerNorm + fused in_proj/depthwise-conv + SiLU with
direction-adjusted taps) and the selective scan are direction-independent
device code. The scan packs rows (d, n) n-minor across partitions (24 tiles
of 128 = 8 d x 16 n), time on the free dim via tensor_tensor_scan;
(d,n)-expansion of delta*A and delta*u is done with bf16 mask-matmuls on
TensorE, exp on ScalarE, n-reduction with accumulating mask-matmuls. The two
direction-dependent reorderings (row-major quarter extraction and the y
un-permute) are uniform 4-candidate strided reads blended with host one-hot
selectors. y is summed across the 4 direction cores with a bf16
ReduceScatter; each core then runs the post-phase (LN -> gate -> out_proj ->
residual -> FFN) on its row-major L-quarter.

Scheduling: near-everything is bf16 (fp32 matmuls are 4 cyc/row vs 1);
the h*C multiply and delta*u run on the otherwise-idle GpSimd engine
(GpSimd cannot read PSUM, so dB*u stays on DVE); conv/x_dbl/dt emission is
interleaved chunk-wise with BOTH scan direction-halves (hj=1 trails hj=0 by
LAG chunks) so the PE-heavy phase-1 overlaps the DVE-bound scan, with the
first ReduceScatter issued mid-scan; the z gate is evaluated only on the
core's own row-major quarter via padded-layout candidate reads (PCAND).
PSUM is the scarce resource: one rotating bank-pool (pp1) serves all
phase-1 matmuls and scan pools are sized to exactly 8 banks total.
"""
import os

import ml_dtypes
import numpy as np
import orjson

# ---------------------------------------------------------------------------
# Workaround: this walrus build caps sync waits at 1 per instruction. Split
# multi-wait instructions in the serialized BIR into preceding same-engine
# NoOps carrying one wait each (same-stream ordering semantics preserved).
# ---------------------------------------------------------------------------


def _split_multiwaits(d):
    n = [0]
    for fn in d.get("functions", []):
        for bb in fn.get("blocks", []):
            out = []
            for inst in bb.get("instructions", []):
                si = inst.get("sync_info")
                waits = (si or {}).get("on_wait") or []
                if len(waits) > 1:
                    for w in waits[:-1]:
                        n[0] += 1
                        out.append({
                            "engine": inst["engine"], "ins": [],
                            "name": f"I-waitsplit-{n[0]}", "opcode": "NoOp",
                            "outs": [],
                            "sync_info": {"on_update": [], "on_wait": [w]},
                        })
                    si["on_wait"] = [waits[-1]]
                out.append(inst)
            bb["instructions"] = out
    return d


def _patch_bass():
    import concourse.bass as bass

    if getattr(bass.Bass.to_json_bytes, "_split_waits", False):
        return
    orig = bass.Bass.to_json_bytes

    def to_json_bytes(self, *a, **k):
        return orjson.dumps(_split_multiwaits(orjson.loads(orig(self, *a, **k))))

    to_json_bytes._split_waits = True
    bass.Bass.to_json_bytes = to_json_bytes


_patch_bass()

import concourse.bass as bass
import concourse.tile as tile
from concourse import mybir
from concourse.bass import AP
from concourse.bass_utils import run_bass_kernel_spmd

F32 = mybir.dt.float32
BF16 = mybir.dt.bfloat16
OP = mybir.AluOpType
AF = mybir.ActivationFunctionType

B, C, H, W = 2, 96, 96, 96
L = H * W              # 9216
DI = 192
NS = 16                # d_state
R = 6                  # dt_rank
HP = H + 2             # 98
LP = HP * HP           # 9604
LC = 384               # chunk length
NCH = L // LC          # 24
LQ = L // 4            # 2304
NT = 24                # scan partition tiles
DEBUG = os.environ.get("BASS_KERNEL_DEBUG", "0") == "1"

# candidate read APs: "row-major quarter q=k" out of a view-ordered [*, L]
# tensor (offsets fixed because quarter q == direction k), and full-L
# un-permute patterns. AP levels are (step, count), outer -> inner.
QCAND = [
    (0, [[1, 2304]]),
    (24, [[1, 24], [96, 96]]),
    (9215 - 4608, [[-96, 24], [-1, 96]]),
    (9215 - 72, [[-1, 24], [-96, 96]]),
]
FCAND = [
    (0, [[1, 9216]]),
    (0, [[1, 96], [96, 96]]),
    (9215, [[-1, 9216]]),
    (9215, [[-1, 96], [-96, 96]]),
]
# padded-layout candidates for "own row-major quarter" read out of xn_pad
PCAND = [
    (99, [[98, 24], [1, 96]]),
    (123, [[1, 24], [98, 96]]),
    (4800, [[-98, 24], [-1, 96]]),
    (9432, [[-1, 24], [-98, 96]]),
]

IN_BF16 = [
    ("amask", [C, NT * 128]), ("repmask", [C, 12 * 128]),
    ("redmask", [128, 12 * C]),
    ("mtap", [C, 9 * DI]), ("w1z", [C, DI]), ("xproj", [C, 2 * 38]),
    ("dtw", [R, DI]), ("bmask", [38, 128]), ("cmask", [38, 128]),
    ("opw", [C, 2 * C]), ("fc1w", [C, DI]), ("fc2w", [C, 2 * C]),
]
IN_SHAPES = [
    ("x_b", [C, L]), ("xq_in", [C, LQ]), ("lng", [C, 1]), ("lnb", [C, 1]),
    ("cbeff", [C, 2]),
    ("b1z", [C, 2]),
    ("dtb", [C, 2]),
    ("sel", [C, 4]), ("dssum", [C, 2]), ("ong", [C, 2]), ("onb", [C, 2]),
    ("opb", [C, 1]), ("s1", [C, 1]),
    ("fng", [C, 1]), ("fnb", [C, 1]), ("f1b", [C, 2]),
    ("f2b", [C, 1]), ("s2", [C, 1]), ("acol", [128, NT]),
]
LCS = 512               # scan chunk length
NCS = L // LCS          # 18


def _ap(t, off, dims):
    return AP(tensor=t.tensor, offset=t.offset + off, ap=[list(t.ap[0])] + dims)


def build_nc():
    import contextlib

    nc = bass.Bass()
    dp = nc.declare_dram_parameter
    ins = {}
    for name, shape in IN_SHAPES:
        ins[name] = dp(name, shape, F32, isOutput=False)
    for name, shape in IN_BF16:
        ins[name] = dp(name, shape, BF16, isOutput=False)
    out_q = dp("out_q", [C, LQ], F32, isOutput=True)
    dbg = {}
    if DEBUG:
        for name, shape in [
            ("xs_dbg", [DI, L]), ("z_dbg", [DI, L]), ("delta_dbg", [DI, L]),
            ("xdbl_dbg", [38, L]), ("yv_dbg", [DI, L]), ("yrm_dbg", [DI, L]),
            ("yt_dbg", [DI, LQ]), ("zq_dbg", [DI, LQ]),
        ]:
            dbg[name] = dp(name, shape, F32, isOutput=True)

    y_dram = [nc.dram_tensor(f"y_dram{h}", [4, C, LQ], BF16) for h in range(2)]
    yrs_dram = [nc.dram_tensor(f"yrs_dram{h}", [C, LQ], BF16) for h in range(2)]
    yview_dram = nc.dram_tensor("yview_dram", [2, C, L], BF16)
    xiq_dram = nc.dram_tensor("xiq_dram", [2, C, LQ], BF16)
    zq_dram = nc.dram_tensor("zq_dram", [2, C, LQ], BF16)

    groups = [[0, 1, 2, 3], [4, 5, 6, 7]]

    with tile.TileContext(nc) as tc:
        tc.race_detector_enabled = False
        with contextlib.ExitStack() as ctx:
            pc = ctx.enter_context(tc.tile_pool(name="pc", bufs=1))
            w = {}
            bf16_names = {n for n, _ in IN_BF16}
            for name in ["lng", "lnb", "cbeff", "b1z", "dtb", "sel", "dssum",
                         "ong", "onb", "opb", "s1", "fng", "fnb", "f1b",
                         "f2b", "s2", "mtap", "w1z", "xproj", "dtw", "opw",
                         "fc1w", "fc2w"]:
                dt = BF16 if name in bf16_names else F32
                t = pc.tile(list(ins[name].shape), dt, tag=name)
                nc.sync.dma_start(out=t, in_=ins[name][:])
                w[name] = t
            ones96 = pc.tile([C, C], F32, tag="ones96")
            nc.vector.memset(ones96, 1.0)
            ones96b = pc.tile([C, C], BF16, tag="ones96b")
            nc.vector.memset(ones96b, 1.0)
            eps6 = pc.tile([C, 1], F32, tag="eps6")
            nc.vector.memset(eps6, 1e-6)
            eps5 = pc.tile([C, 1], F32, tag="eps5")
            nc.vector.memset(eps5, 1e-5)

            # ---- scan pools (shared PSUM with phase-1 matmuls) -----------
            with tc.tile_pool(name="pxd", bufs=1) as pxd, \
                 tc.tile_pool(name="pxs", bufs=1) as pxs, \
                 tc.tile_pool(name="pdl", bufs=1) as pdl, \
                 tc.tile_pool(name="pm", bufs=1) as pm, \
                 tc.tile_pool(name="pscan", bufs=1) as pscan, \
                 tc.tile_pool(name="psw", bufs=4) as psw, \
                 tc.tile_pool(name="psd", bufs=2) as psd, \
                 tc.tile_pool(name="pp1", bufs=2, space="PSUM") as pp1, \
                 tc.tile_pool(name="ppau", bufs=3, space="PSUM") as ppau, \
                 tc.tile_pool(name="ppbc", bufs=1, space="PSUM") as ppbc, \
                 tc.tile_pool(name="ppy", bufs=2, space="PSUM") as ppy:
                xdbl = pxd.tile([38, L], BF16, tag="xdbl")
                xs = [pxs.tile([C, L], BF16, tag=f"xs{h}", name=f"xs{h}")
                      for h in range(2)]
                dlsb = [pdl.tile([C, L], BF16, tag=f"dl{h}", name=f"dl{h}")
                        for h in range(2)]
                amask = pm.tile([C, NT * 128], BF16, tag="amask")
                repmask = pm.tile([C, 12 * 128], BF16, tag="repmask")
                bmask = pm.tile([38, 128], BF16, tag="bmask")
                cmask = pm.tile([38, 128], BF16, tag="cmask")
                redmask = pm.tile([128, 12 * C], BF16, tag="redmask")
                for t, name in [(amask, "amask"), (repmask, "repmask"),
                                (bmask, "bmask"), (cmask, "cmask"),
                                (redmask, "redmask")]:
                    nc.sync.dma_start(out=t, in_=ins[name][:])
                h_tiles = [pscan.tile([128, LCS], BF16, tag=f"h{j}", name=f"h{j}")
                           for j in range(NT)]

                def emit_pre(hj, i):
                    # bb/cb/du for chunk (hj, i), emitted a chunk early so
                    # they clear the ACT/DVE queues before the body needs them
                    ci = LCS * i
                    ps_b = ppbc.tile([128, LCS], F32, tag="bc_ps")
                    nc.tensor.matmul(ps_b[:], bmask[:], xdbl[:, ci:ci + LCS],
                                     start=True, stop=True)
                    bb_t = psw.tile([128, LCS], BF16, tag="bb")
                    nc.scalar.copy(bb_t[:], ps_b[:])
                    ps_c = ppbc.tile([128, LCS], F32, tag="bc_ps")
                    nc.tensor.matmul(ps_c[:], cmask[:], xdbl[:, ci:ci + LCS],
                                     start=True, stop=True)
                    cb_t = psw.tile([128, LCS], BF16, tag="cb")
                    nc.scalar.copy(cb_t[:], ps_c[:])
                    du_t = psd.tile([C, LCS], BF16, tag=f"du{hj}")
                    # hj=0's du_t is on the fresh p1-chunk's critical path:
                    # don't queue it behind a chunk's worth of gpsimd hc ops
                    dueng = nc.vector if hj == 0 else nc.gpsimd
                    dueng.tensor_tensor(
                        du_t[:], dlsb[hj][:, ci:ci + LCS],
                        xs[hj][:, ci:ci + LCS], op=OP.mult)
                    return bb_t, cb_t, du_t

                def emit_scan_chunk(hj, i, pre):
                    ci = LCS * i
                    bb_t, cb_t, du_t = pre
                    dl = dlsb[hj][:, ci:ci + LCS]
                    ps_y = ppy.tile([C, LCS], F32, tag="y_ps")
                    for jj in range(12):
                        j = 12 * hj + jj
                        ps_a = ppau.tile([128, LCS], F32, tag="au_ps")
                        nc.tensor.matmul(
                            ps_a[:], amask[:, 128 * j:128 * j + 128],
                            dl, start=True, stop=True)
                        a_t = psw.tile([128, LCS], BF16, tag="a_t")
                        nc.scalar.activation(a_t[:], ps_a[:], AF.Exp)
                        ps_u = ppau.tile([128, LCS], F32, tag="au_ps")
                        nc.tensor.matmul(
                            ps_u[:], repmask[:, 128 * jj:128 * jj + 128],
                            du_t[:], start=True, stop=True)
                        dbu_t = psw.tile([128, LCS], BF16, tag="dbu")
                        nc.vector.tensor_tensor(dbu_t[:], ps_u[:],
                                                bb_t[:], op=OP.mult)
                        hj_t = h_tiles[j]
                        init = 0.0 if i == 0 else hj_t[:, LCS - 1:LCS]
                        nc.vector.tensor_tensor_scan(
                            hj_t[:], a_t[:], dbu_t[:], init,
                            op0=OP.mult, op1=OP.add)
                        hc_t = psw.tile([128, LCS], BF16, tag="hc")
                        nc.gpsimd.tensor_tensor(hc_t[:], hj_t[:], cb_t[:],
                                                op=OP.mult)
                        nc.tensor.matmul(
                            ps_y[:], redmask[:, C * jj:C * jj + C],
                            hc_t[:], start=(jj == 0), stop=(jj == 11))
                    yb_t = psw.tile([C, LCS], BF16, tag="yb")
                    nc.scalar.copy(yb_t[:], ps_y[:])
                    nc.sync.dma_start(out=yview_dram[hj][:, ci:ci + LCS],
                                      in_=yb_t[:])

                def emit_unpermute_rs(hj, yv, yrm):
                    nc.sync.dma_start(out=yv[:, 0:L], in_=yview_dram[hj][:])
                    for k in range(4):
                        off, dims = FCAND[k]
                        if k == 0:
                            nc.vector.tensor_scalar(
                                yrm[:, 0:L], _ap(yv, off, dims),
                                w["sel"][:, k:k + 1], None, op0=OP.mult)
                        else:
                            nc.vector.scalar_tensor_tensor(
                                yrm[:, 0:L], _ap(yv, off, dims),
                                w["sel"][:, k:k + 1], yrm[:, 0:L],
                                op0=OP.mult, op1=OP.add)
                    for q in range(4):
                        nc.sync.dma_start(out=y_dram[hj][q],
                                          in_=yrm[:, LQ * q:LQ * q + LQ])
                    nc.gpsimd.collective_compute(
                        "ReduceScatter", OP.add,
                        ins=[y_dram[hj][:]], outs=[yrs_dram[hj][:]],
                        replica_groups=groups)

                with tc.tile_pool(name="ppad", bufs=1) as ppad:
                    xn_pad = ppad.tile([C, LP], BF16, tag="xn_pad")
                    nc.vector.memset(xn_pad, 0.0)

                    def emit_ln(i, pw):
                        ci = LC * i
                        h0 = 4 * i
                        xc_t = pw.tile([C, LC], F32, tag="x_c")
                        nc.sync.dma_start(out=xc_t,
                                          in_=ins["x_b"][:, ci:ci + LC])
                        xc = xc_t[:]
                        xcb = pw.tile([C, LC], BF16, tag="xcb")
                        nc.scalar.copy(xcb[:], xc)
                        sq = pw.tile([C, LC], BF16, tag="t1b")
                        nc.gpsimd.tensor_tensor(sq[:], xc, xc, op=OP.mult)
                        ps_s = pp1.tile([C, LC], F32, tag="p1")
                        ps_q = pp1.tile([C, LC], F32, tag="p1")
                        nc.tensor.matmul(ps_s[:], ones96b[:], xcb[:],
                                         start=True, stop=True)
                        nc.tensor.matmul(ps_q[:], ones96b[:], sq[:],
                                         start=True, stop=True)
                        m_t = pw.tile([C, LC], F32, tag="m")
                        msq = pw.tile([C, LC], F32, tag="msq")
                        nc.scalar.activation(m_t[:], ps_s[:], AF.Copy,
                                             scale=1.0 / 96)
                        nc.scalar.activation(msq[:], ps_s[:], AF.Square,
                                             scale=1.0 / 96)
                        var = pw.tile([C, LC], F32, tag="var")
                        nc.vector.scalar_tensor_tensor(
                            var[:], ps_q[:], 1.0 / 96, msq[:],
                            op0=OP.mult, op1=OP.subtract)
                        lnv = pw.tile([C, LC], F32, tag="var")
                        nc.scalar.activation(lnv[:], var[:], AF.Ln,
                                             bias=eps6[:])
                        rstd = pw.tile([C, LC], F32, tag="rstd")
                        nc.scalar.activation(rstd[:], lnv[:], AF.Exp,
                                             scale=-0.5)
                        d1 = pw.tile([C, LC], F32, tag="d1")
                        nc.gpsimd.tensor_tensor(d1[:], xc, m_t[:],
                                                op=OP.subtract)
                        d2 = pw.tile([C, LC], F32, tag="d2")
                        nc.gpsimd.tensor_tensor(d2[:], d1[:], rstd[:],
                                                op=OP.mult)
                        outap = _ap(xn_pad, (h0 + 1) * HP + 1,
                                    [[HP, 4], [1, W]])
                        nc.vector.tensor_scalar(
                            outap, d2[:], w["lng"][:], w["lnb"][:],
                            op0=OP.mult, op1=OP.add)

                    def emit_z():
                        # z gate on own row-major quarter only (lag window)
                        with tc.tile_pool(name="pzq", bufs=1) as pzq:
                            xnq = pzq.tile([C, LQ], BF16, tag="xnq",
                                           name="xnq")
                            for k in range(4):
                                off, dims = PCAND[k]
                                if k == 0:
                                    nc.vector.tensor_scalar(
                                        xnq[:], _ap(xn_pad, off, dims),
                                        w["sel"][:, k:k + 1], None,
                                        op0=OP.mult)
                                else:
                                    nc.vector.scalar_tensor_tensor(
                                        xnq[:], _ap(xn_pad, off, dims),
                                        w["sel"][:, k:k + 1], xnq[:],
                                        op0=OP.mult, op1=OP.add)
                            for hf in range(2):
                                for i in range(LQ // LC):
                                    ci = LC * i
                                    ps_z = pp1.tile([C, LC], F32, tag="p1")
                                    nc.tensor.matmul(
                                        ps_z[:],
                                        w["w1z"][:, C * hf:C * hf + C],
                                        xnq[:, ci:ci + LC],
                                        start=True, stop=True)
                                    zq_c = psd.tile([C, LC], BF16, tag="zq_c")
                                    nc.scalar.activation(
                                        zq_c[:], ps_z[:],
                                        AF.Silu, bias=w["b1z"][:, hf:hf + 1])
                                    nc.sync.dma_start(
                                        out=zq_dram[hf][:, ci:ci + LC],
                                        in_=zq_c[:])

                    with tc.tile_pool(name="pln", bufs=2) as pw_ln:
                        for i in range(NCH):
                            emit_ln(i, pw_ln)
                    emit_z()

                    # ---- fused: conv/x_dbl/delta feed the hj=0 scan ------
                    def emit_p1_chunk(i):
                        ci = LC * i
                        h0 = 4 * i
                        for hf in range(2):
                            ps = pp1.tile([C, LC], F32, tag="p1")
                            for tap in range(9):
                                ti, tj = tap // 3, tap % 3
                                rhs = _ap(xn_pad, (h0 + ti) * HP + tj,
                                          [[HP, 4], [1, W]])
                                nc.tensor.matmul(
                                    ps[:],
                                    w["mtap"][:, tap * DI + C * hf:
                                              tap * DI + C * hf + C],
                                    rhs, start=(tap == 0), stop=(tap == 8))
                            nc.scalar.activation(
                                xs[hf][:, ci:ci + LC], ps[:], AF.Silu,
                                bias=w["cbeff"][:, hf:hf + 1])
                        ps = pp1.tile([38, LC], F32, tag="p1")
                        nc.tensor.matmul(ps[:], w["xproj"][:, 0:38],
                                         xs[0][:, ci:ci + LC],
                                         start=True, stop=False)
                        nc.tensor.matmul(ps[:], w["xproj"][:, 38:76],
                                         xs[1][:, ci:ci + LC],
                                         start=False, stop=True)
                        nc.scalar.copy(xdbl[:, ci:ci + LC], ps[:])
                        for hf in range(2):
                            ps = pp1.tile([C, LC], F32, tag="p1")
                            nc.tensor.matmul(
                                ps[:], w["dtw"][:, C * hf:C * hf + C],
                                xdbl[0:R, ci:ci + LC], start=True, stop=True)
                            e_t = psd.tile([C, LC], F32, tag="dt_e")
                            nc.scalar.activation(e_t[:], ps[:], AF.Exp,
                                                 bias=w["dtb"][:, hf:hf + 1])
                            nc.scalar.activation(
                                dlsb[hf][:, ci:ci + LC], e_t[:], AF.Ln,
                                bias=1.0)

                    # hj=1 scan chunks trail hj=0 by LAG so RS0 overlaps
                    LAG = 3
                    jdone = 0
                    pre0 = pre1 = None
                    for t in range(NCS + LAG):
                        if t < NCS:
                            # emit p1 two scan-chunks ahead of first use so
                            # the softplus ACT chain clears the queue early
                            jt = min(NCH, -(-(LCS * (t + 3)) // LC))
                            while jdone < jt:
                                emit_p1_chunk(jdone)
                                jdone += 1
                        if t < NCS:
                            if pre0 is None:
                                pre0 = emit_pre(0, t)
                            emit_scan_chunk(0, t, pre0)
                            pre0 = (emit_pre(0, t + 1)
                                    if t + 1 < NCS else None)
                        if t >= LAG:
                            i1 = t - LAG
                            if pre1 is None:
                                pre1 = emit_pre(1, i1)
                            emit_scan_chunk(1, i1, pre1)
                            pre1 = (emit_pre(1, i1 + 1)
                                    if i1 + 1 < NCS else None)
                        if t == NCS - 1:
                            # xn_pad and dlsb[0] are dead here; reuse space
                            emit_unpermute_rs(0, xn_pad, dlsb[0])

                # ---- un-permute + RS for hj=1; xs quarter extract --------
                with tc.tile_pool(name="pyv", bufs=1) as pyv:
                    yv1 = pyv.tile([C, L], BF16, tag="yv")
                    yrm1 = pyv.tile([C, L], BF16, tag="yrm")
                    emit_unpermute_rs(1, yv1, yrm1)
                    with tc.tile_pool(name="pq1", bufs=1) as pq:
                        for hf in range(2):
                            acc = pq.tile([C, LQ], BF16, tag="q_acc")
                            nc.vector.memset(acc, 0.0)
                            for k in range(4):
                                off, dims = QCAND[k]
                                nc.vector.scalar_tensor_tensor(
                                    acc[:], _ap(xs[hf], off, dims),
                                    w["sel"][:, k:k + 1], acc[:],
                                    op0=OP.mult, op1=OP.add)
                            nc.sync.dma_start(out=xiq_dram[hf], in_=acc[:])

            # ---- post-phase on the row-major quarter ---------------------
            with tc.tile_pool(name="ppo", bufs=1) as po, \
                 tc.tile_pool(name="pow", bufs=2) as pw, \
                 tc.tile_pool(name="pop", bufs=2, space="PSUM") as pp, \
                 tc.tile_pool(name="pop2", bufs=1, space="PSUM") as pp2:
                yt = [po.tile([C, LQ], BF16, tag=f"yt{h}", name=f"yt{h}") for h in range(2)]
                zq = [po.tile([C, LQ], BF16, tag=f"zq{h}", name=f"zq{h}") for h in range(2)]
                x2 = po.tile([C, LQ], BF16, tag="x2")
                xq = po.tile([C, LQ], F32, tag="xq")
                out_sb = po.tile([C, LQ], F32, tag="out_sb")
                nc.sync.dma_start(out=xq, in_=ins["xq_in"][:])
                for hf in range(2):
                    yrs_t = pw.tile([C, LQ], BF16, tag="yrs_t")
                    nc.sync.dma_start(out=yrs_t, in_=yrs_dram[hf][:])
                    xiq_t = pw.tile([C, LQ], BF16, tag="xiq_t")
                    nc.sync.dma_start(out=xiq_t, in_=xiq_dram[hf])
                    nc.sync.dma_start(out=zq[hf], in_=zq_dram[hf])
                    nc.vector.scalar_tensor_tensor(
                        yt[hf][:], xiq_t[:], w["dssum"][:, hf:hf + 1],
                        yrs_t[:], op0=OP.mult, op1=OP.add)
                if DEBUG:
                    for hf in range(2):
                        nc.sync.dma_start(
                            out=dbg["yt_dbg"][C * hf:C * hf + C, :],
                            in_=yt[hf][:])
                        nc.sync.dma_start(
                            out=dbg["zq_dbg"][C * hf:C * hf + C, :],
                            in_=zq[hf][:])

                NPQ = LQ // LC
                with tc.tile_pool(name="pst", bufs=1) as pst:
                    mt_s = [pst.tile([C, LC], F32, tag=f"m{i}", name=f"m{i}")
                            for i in range(NPQ)]
                    rs_s = [pst.tile([C, LC], F32, tag=f"r{i}", name=f"r{i}")
                            for i in range(NPQ)]
                    fm_s = [pst.tile([C, LC], F32, tag=f"fm{i}",
                                     name=f"fm{i}") for i in range(NPQ)]
                    fr_s = [pst.tile([C, LC], F32, tag=f"fr{i}",
                                     name=f"fr{i}") for i in range(NPQ)]
                    # sweep 1: out-LN stats for all chunks
                    for i in range(NPQ):
                        ci = LC * i
                        ps_s = pp.tile([C, LC], F32, tag="po_s")
                        ps_q = pp.tile([C, LC], F32, tag="po_q")
                        for hf in range(2):
                            ytc = yt[hf][:, ci:ci + LC]
                            sq = pw.tile([C, LC], BF16, tag=f"sq{hf}")
                            nc.gpsimd.tensor_tensor(sq[:], ytc, ytc,
                                                    op=OP.mult)
                            nc.tensor.matmul(ps_s[:], ones96b[:], ytc,
                                             start=(hf == 0), stop=(hf == 1))
                            nc.tensor.matmul(ps_q[:], ones96b[:], sq[:],
                                             start=(hf == 0), stop=(hf == 1))
                        msq = pw.tile([C, LC], F32, tag="msq")
                        nc.scalar.activation(mt_s[i][:], ps_s[:], AF.Copy,
                                             scale=1.0 / DI)
                        nc.scalar.activation(msq[:], ps_s[:], AF.Square,
                                             scale=1.0 / DI)
                        var = pw.tile([C, LC], F32, tag="var")
                        nc.vector.scalar_tensor_tensor(
                            var[:], ps_q[:], 1.0 / DI, msq[:],
                            op0=OP.mult, op1=OP.subtract)
                        lnv = pw.tile([C, LC], F32, tag="lnv")
                        nc.scalar.activation(lnv[:], var[:], AF.Ln,
                                             bias=eps5[:])
                        nc.scalar.activation(rs_s[i][:], lnv[:], AF.Exp,
                                             scale=-0.5)
                    # sweep 2: out-LN apply, gate, out_proj, x2, FFN stats
                    for i in range(NPQ):
                        ci = LC * i
                        ps_o = pp2.tile([C, LC], F32, tag="po_o")
                        for hf in range(2):
                            ytc = yt[hf][:, ci:ci + LC]
                            d1 = pw.tile([C, LC], F32, tag=f"d1{hf}")
                            nc.gpsimd.tensor_tensor(d1[:], ytc, mt_s[i][:],
                                                    op=OP.subtract)
                            d2 = pw.tile([C, LC], F32, tag=f"d2{hf}")
                            nc.vector.tensor_tensor(d2[:], d1[:], rs_s[i][:],
                                                    op=OP.mult)
                            yn = pw.tile([C, LC], F32, tag=f"yn{hf}")
                            nc.vector.tensor_scalar(
                                yn[:], d2[:], w["ong"][:, hf:hf + 1],
                                w["onb"][:, hf:hf + 1],
                                op0=OP.mult, op1=OP.add)
                            g_t = pw.tile([C, LC], BF16, tag=f"g{hf}")
                            nc.vector.tensor_tensor(g_t[:], yn[:],
                                                    zq[hf][:, ci:ci + LC],
                                                    op=OP.mult)
                            nc.tensor.matmul(ps_o[:],
                                             w["opw"][:, C * hf:C * hf + C],
                                             g_t[:], start=(hf == 0),
                                             stop=(hf == 1))
                        ss = pw.tile([C, LC], F32, tag="ss")
                        nc.scalar.activation(ss[:], ps_o[:], AF.Identity,
                                             bias=w["opb"][:])
                        nc.vector.scalar_tensor_tensor(
                            x2[:, ci:ci + LC], xq[:, ci:ci + LC], w["s1"][:],
                            ss[:], op0=OP.mult, op1=OP.add)
                        x2c = x2[:, ci:ci + LC]
                        sq2 = pw.tile([C, LC], BF16, tag="sq2")
                        nc.gpsimd.tensor_tensor(sq2[:], x2c, x2c, op=OP.mult)
                        ps_fs = pp.tile([C, LC], F32, tag="po_s")
                        ps_fq = pp.tile([C, LC], F32, tag="po_q")
                        nc.tensor.matmul(ps_fs[:], ones96b[:], x2c,
                                         start=True, stop=True)
                        nc.tensor.matmul(ps_fq[:], ones96b[:], sq2[:],
                                         start=True, stop=True)
                        fmsq = pw.tile([C, LC], F32, tag="fmsq")
                        nc.scalar.activation(fm_s[i][:], ps_fs[:], AF.Copy,
                                             scale=1.0 / 96)
                        nc.scalar.activation(fmsq[:], ps_fs[:], AF.Square,
                                             scale=1.0 / 96)
                        fvar = pw.tile([C, LC], F32, tag="fvar")
                        nc.vector.scalar_tensor_tensor(
                            fvar[:], ps_fq[:], 1.0 / 96, fmsq[:],
                            op0=OP.mult, op1=OP.subtract)
                        flnv = pw.tile([C, LC], F32, tag="flnv")
                        nc.scalar.activation(flnv[:], fvar[:], AF.Ln,
                                             bias=eps5[:])
                        nc.scalar.activation(fr_s[i][:], flnv[:], AF.Exp,
                                             scale=-0.5)
                    # sweep 3: FFN apply
                    for i in range(NPQ):
                        ci = LC * i
                        x2c = x2[:, ci:ci + LC]
                        fd1 = pw.tile([C, LC], F32, tag="fd1")
                        nc.gpsimd.tensor_tensor(fd1[:], x2c, fm_s[i][:],
                                                op=OP.subtract)
                        fd2 = pw.tile([C, LC], F32, tag="fd2")
                        nc.vector.tensor_tensor(fd2[:], fd1[:], fr_s[i][:],
                                                op=OP.mult)
                        hn = pw.tile([C, LC], BF16, tag="hn")
                        nc.vector.tensor_scalar(
                            hn[:], fd2[:], w["fng"][:], w["fnb"][:],
                            op0=OP.mult, op1=OP.add)
                        h1 = []
                        for hf in range(2):
                            ps_f = pp2.tile([C, LC], F32, tag=f"ff_f{hf}")
                            nc.tensor.matmul(ps_f[:],
                                             w["fc1w"][:, C * hf:C * hf + C],
                                             hn[:], start=True, stop=True)
                            h1t = pw.tile([C, LC], BF16, tag=f"h1{hf}b")
                            nc.scalar.activation(h1t[:], ps_f[:], AF.Prelu,
                                                 bias=w["f1b"][:, hf:hf + 1],
                                                 alpha=0.01)
                            h1.append(h1t)
                        ps_f2 = pp.tile([C, LC], F32, tag="po_s")
                        for hf in range(2):
                            nc.tensor.matmul(ps_f2[:],
                                             w["fc2w"][:, C * hf:C * hf + C],
                                             h1[hf][:], start=(hf == 0),
                                             stop=(hf == 1))
                        ff_t = pw.tile([C, LC], F32, tag="ff")
                        nc.scalar.activation(ff_t[:], ps_f2[:], AF.Identity,
                                             bias=w["f2b"][:])
                        nc.vector.scalar_tensor_tensor(
                            out_sb[:, ci:ci + LC], x2c, w["s2"][:], ff_t[:],
                            op0=OP.mult, op1=OP.add)
                nc.sync.dma_start(out=out_q[:], in_=out_sb[:])

    return nc


def _prep_inputs(inputs):
    x = np.asarray(inputs["x"], np.float32)
    in_proj_w = np.asarray(inputs["in_proj_w"], np.float32)
    in_proj_b = np.asarray(inputs["in_proj_b"], np.float32)
    conv_w = np.asarray(inputs["conv_w"], np.float32)
    conv_b = np.asarray(inputs["conv_b"], np.float32)
    x_proj_w = np.asarray(inputs["x_proj_w"], np.float32)
    dt_w = np.asarray(inputs["dt_w"], np.float32)
    dt_b = np.asarray(inputs["dt_b"], np.float32)
    A_logs = np.asarray(inputs["A_logs"], np.float32)
    Ds = np.asarray(inputs["Ds"], np.float32)
    out_proj_w = np.asarray(inputs["out_proj_w"], np.float32)
    fc2_w = np.asarray(inputs["fc2_w"], np.float32)

    A = -np.exp(A_logs)
    dssum = Ds.sum(axis=0)
    W1x = in_proj_w[:DI]
    W1z = in_proj_w[DI:]
    b1x = in_proj_b[:DI]
    b1z_v = in_proj_b[DI:]
    cw = conv_w[:, 0]
    cbeff = conv_b + b1x * cw.sum(axis=(1, 2))

    repmask = np.zeros((12, C, 128), np.float32)
    for q in range(12):
        for ds in range(8):
            repmask[q, 8 * q + ds, 16 * ds:16 * ds + 16] = 1.0
    bmask = np.zeros((38, 128), np.float32)
    cmask = np.zeros((38, 128), np.float32)
    for ds in range(8):
        for n in range(NS):
            bmask[R + n, 16 * ds + n] = 1.0
            cmask[R + NS + n, 16 * ds + n] = 1.0
    redmask = np.zeros((12, 128, C), np.float32)
    for q in range(12):
        for ds in range(8):
            for n in range(NS):
                redmask[q, 16 * ds + n, 8 * q + ds] = 1.0

    def col2(v):
        return np.ascontiguousarray(np.asarray(v, np.float32).reshape(2, C).T)

    in_maps = []
    for c in range(8):
        b, k = c // 4, c % 4
        x2d = x[b].reshape(C, H, W)
        if k == 0:
            xb = x2d.reshape(C, L)
        elif k == 1:
            xb = x2d.transpose(0, 2, 1).reshape(C, L)
        elif k == 2:
            xb = x2d.reshape(C, L)[:, ::-1]
        else:
            xb = x2d.transpose(0, 2, 1).reshape(C, L)[:, ::-1]
        xq = x2d.reshape(C, L)[:, LQ * k:LQ * k + LQ]

        mtap = np.zeros((C, 9 * DI), np.float32)
        for ti in range(3):
            for tj in range(3):
                if k == 0:
                    oi, oj = ti, tj
                elif k == 1:
                    oi, oj = tj, ti
                elif k == 2:
                    oi, oj = 2 - ti, 2 - tj
                else:
                    oi, oj = 2 - tj, 2 - ti
                m = W1x * cw[:, oi, oj][:, None]
                mtap[:, (ti * 3 + tj) * DI:(ti * 3 + tj + 1) * DI] = m.T

        acol = np.zeros((128, NT), np.float32)
        amask = np.zeros((NT, C, 128), np.float32)
        for j in range(NT):
            for ds in range(8):
                acol[16 * ds:16 * ds + 16, j] = A[k, 8 * j + ds]
                d = 8 * j + ds
                amask[j, d % C, 16 * ds:16 * ds + 16] = A[k, d]
        sel = np.zeros((C, 4), np.float32)
        sel[:, k] = 1.0

        xproj = np.zeros((C, 76), np.float32)
        opw = np.zeros((C, 2 * C), np.float32)
        fc2w = np.zeros((C, 2 * C), np.float32)
        for hf in range(2):
            xproj[:, 38 * hf:38 * hf + 38] = x_proj_w[k, :, C * hf:C * hf + C].T
            opw[:, C * hf:C * hf + C] = out_proj_w[:, C * hf:C * hf + C].T
            fc2w[:, C * hf:C * hf + C] = fc2_w[:, C * hf:C * hf + C].T

        bf = ml_dtypes.bfloat16
        im = {
            "x_b": np.ascontiguousarray(xb),
            "xq_in": np.ascontiguousarray(xq),
            "lng": np.asarray(inputs["ln_in_g"], np.float32).reshape(C, 1),
            "lnb": np.asarray(inputs["ln_in_b"], np.float32).reshape(C, 1),
            "mtap": mtap.astype(bf),
            "cbeff": col2(cbeff),
            "w1z": np.ascontiguousarray(W1z.T).astype(bf),
            "b1z": col2(b1z_v),
            "xproj": xproj.astype(bf),
            "dtw": np.ascontiguousarray(dt_w[k].T).astype(bf),
            "dtb": col2(dt_b[k]),
            "acol": acol,
            "amask": np.ascontiguousarray(
                amask.transpose(1, 0, 2).reshape(C, NT * 128)).astype(bf),
            "repmask": np.ascontiguousarray(
                repmask.transpose(1, 0, 2).reshape(C, 12 * 128)).astype(ml_dtypes.bfloat16),
            "bmask": bmask.astype(bf),
            "cmask": cmask.astype(bf),
            "redmask": np.ascontiguousarray(
                redmask.transpose(1, 0, 2).reshape(128, 12 * C)).astype(ml_dtypes.bfloat16),
            "sel": sel,
            "dssum": col2(dssum),
            "ong": col2(inputs["out_norm_g"]),
            "onb": col2(inputs["out_norm_b"]),
            "opw": opw.astype(bf),
            "opb": np.asarray(inputs["out_proj_b"], np.float32).reshape(C, 1),
            "s1": np.asarray(inputs["scale1"], np.float32).reshape(C, 1),
            "fng": np.asarray(inputs["ln_ffn_g"], np.float32).reshape(C, 1),
            "fnb": np.asarray(inputs["ln_ffn_b"], np.float32).reshape(C, 1),
            "fc1w": np.ascontiguousarray(
                np.asarray(inputs["fc1_w"], np.float32).T).astype(bf),
            "f1b": col2(inputs["fc1_b"]),
            "fc2w": fc2w.astype(bf),
            "f2b": np.asarray(inputs["fc2_b"], np.float32).reshape(C, 1),
            "s2": np.asarray(inputs["scale2"], np.float32).reshape(C, 1),
        }
        in_maps.append(im)
    return in_maps


_NC_CACHE = [None]


def kernel(**inputs):
    if _NC_CACHE[0] is None:
        _NC_CACHE[0] = build_nc()
    nc = _NC_CACHE[0]
    in_maps = _prep_inputs(inputs)
    trace = os.environ.get("BASS_KERNEL_TRACE", "0") == "1"
    res = run_bass_kernel_spmd(nc, in_maps, core_ids=list(range(8)),
                               trace=trace)
    out = np.zeros((B, C, H, W), np.float32)
    for c in range(8):
        b, k = c // 4, c % 4
        oq = res.results[c]["out_q"]
        out[b, :, 24 * k:24 * k + 24, :] = oq.reshape(C, 24, W)
    kernel.last_results = res
    return out


if __name__ == "__main__":
    import reference

    inputs = {k: np.asarray(v) for k, v in reference.setup_inputs().items()}
    out = kernel(**inputs)
    exp = np.asarray(reference.reference(**reference.setup_inputs()))
    err = np.abs(out - exp).max() / (np.abs(exp).max() + 1e-12)
    print("rel err:", err)

